# revision 30
# baseline (speedup 1.0000x reference)
"""BCE survival loss on 8 trn2 NeuronCores.

Math (per row i of preds [N,T], d=clip(targets_d,0,T-1), e=targets_e!=0):
  yth = d + (1-e)            # y[i,j] = [j < yth]   (bce "target" prefix)
  mth = e ? T : d+1          # mask[i,j] = [j < mth]
  bce = softplus(x) - y*x    # == -(y*log S + (1-y)*log1p(-S)) for S=sigmoid(x)
  per_sample = sum_j w_j*mask*(softplus(x) - y*x) / mth
  out = sum_i sw_i*per_sample_i / max(sum_i sw_i, eps)

Let alpha_i = sw_i/mth_i, kA_i = e?T-1:d (mask prefix end, inclusive),
kS_i = d-e (y prefix end, inclusive; -1 => empty). Then

  NUM = sum_j w_j * ( G1[j,j] - G2[j,j] )
  G1[j,k] = sum_i alpha_i*[k<=kA_i]*softplus(x_ij)   (k==j slice used)
  G2[j,k] = sum_i alpha_i*[k<=kS_i]*x_ij

G1/G2 are computed as PSUM-accumulated matmuls over 128-row blocks:
stationary = data block [128 rows, T], moving = per-row prefix matrix
[128 rows, T] built by one tensor_scalar (is_le, mult) per block.
Row r of a shard maps to (partition, block) = (r // 128, r % 128) so the
per-block scalar vectors are just columns of the naturally-loaded
[128,128] metadata tiles (no transposes anywhere).

Only the diagonal of G1/G2 is used; host does the final tiny reduction.
"""

import os
from contextlib import ExitStack

import numpy as np

import concourse.bacc as bacc
import concourse.bass as bass
import concourse.mybir as mybir
import concourse.tile as tile
from concourse.bass_utils import run_bass_kernel_spmd

dt = mybir.dt
Alu = mybir.AluOpType

N, T = 131072, 128
NCORES = 8
NS = N // NCORES          # rows per core shard = 16384
BLOCKS = NS // 128        # 128 row-blocks per core
SUPER = 16                # blocks per super-tile (DMA/ACT granularity)
NSUP = BLOCKS // SUPER    # 8 super-tiles
EPS = 1e-9

LAST_RESULTS = None       # BassKernelResults of the most recent run (for test.py)


def build_program(mb=None):
    """mb: per-block matmul/mask column extents (len BLOCKS, descending,
    multiples of 8, mb[0]==T). Rows are host-sorted descending by mask
    extent so block b only needs columns [0, mb[b])."""
    if mb is None:
        mb = (T,) * BLOCKS
    nc = bacc.Bacc(
        "TRN2", target_bir_lowering=False, debug=False, num_devices=NCORES
    )
    preds = nc.dram_tensor("preds", [NS, T], dt.float32, kind="ExternalInput").ap()
    d_in = nc.dram_tensor("d", [128, BLOCKS], dt.int32, kind="ExternalInput").ap()
    e_in = nc.dram_tensor("e", [128, BLOCKS], dt.int32, kind="ExternalInput").ap()
    sw_in = nc.dram_tensor("sw", [128, BLOCKS], dt.float32, kind="ExternalInput").ap()
    g1_out = nc.dram_tensor("g1", [128, T], dt.float32, kind="ExternalOutput").ap()
    g2_out = nc.dram_tensor("g2", [128, T], dt.float32, kind="ExternalOutput").ap()

    # preds[p*128 + b, t] viewed as [p, b, t]
    preds3 = preds.rearrange("(p b) t -> p b t", p=128)

    with ExitStack() as ctx:
        tc = ctx.enter_context(tile.TileContext(nc))
        xpool = ctx.enter_context(tc.tile_pool(name="x", bufs=4))
        spool = ctx.enter_context(tc.tile_pool(name="sp", bufs=4))
        # one pfx buffer per block and tag: buffers are never reused, so
        # TileContext emits no WAR EventSemaphore wait instructions on the
        # DVE SEQ (~70ns of serial dispatch each, 2 per block otherwise)
        ppool = ctx.enter_context(tc.tile_pool(name="pfx", bufs=BLOCKS + 1))
        meta = ctx.enter_context(tc.tile_pool(name="meta", bufs=1))
        psum = ctx.enter_context(tc.tile_pool(name="acc", bufs=1, space="PSUM"))

        # ---- one-time prep (metadata via the Pool SWDGE queue so the x
        # loads own the HWDGE path from t=0) ----
        d_t = meta.tile([128, BLOCKS], dt.int32, tag="d_t")
        nc.gpsimd.dma_start(d_t[:], d_in)
        e_t = meta.tile([128, BLOCKS], dt.int32, tag="e_t")
        nc.gpsimd.dma_start(e_t[:], e_in)
        sw_t = meta.tile([128, BLOCKS], dt.float32, tag="sw_t")
        nc.gpsimd.dma_start(sw_t[:], sw_in)

        df = meta.tile([128, BLOCKS], dt.float32, tag="df")
        nc.vector.tensor_copy(df[:], d_t[:])
        ef = meta.tile([128, BLOCKS], dt.float32, tag="ef")
        nc.vector.tensor_copy(ef[:], e_t[:])

        # tsum = d + 200*e ; kA = min(tsum,127) ; mth = min(tsum+1,128) ; kS = d-e
        tsum = meta.tile([128, BLOCKS], dt.float32, tag="tsum")
        nc.vector.tensor_scalar(tsum[:], ef[:], 200.0, None, Alu.mult)
        nc.vector.tensor_add(tsum[:], tsum[:], df[:])
        kA = meta.tile([128, BLOCKS], dt.float32, tag="kA")
        nc.vector.tensor_scalar(kA[:], tsum[:], 127.0, None, Alu.min)
        mth = meta.tile([128, BLOCKS], dt.float32, tag="mth")
        nc.vector.tensor_scalar(mth[:], tsum[:], 1.0, 128.0, Alu.add, Alu.min)
        kS = meta.tile([128, BLOCKS], dt.float32, tag="kS")
        nc.vector.tensor_sub(kS[:], df[:], ef[:])
        rec = meta.tile([128, BLOCKS], dt.float32, tag="rec")
        nc.vector.reciprocal(rec[:], mth[:])
        alpha = meta.tile([128, BLOCKS], dt.float32, tag="alpha")
        nc.vector.tensor_mul(alpha[:], sw_t[:], rec[:])

        iota_bf = meta.tile([128, T], dt.bfloat16, tag="iota_bf")
        nc.gpsimd.iota(
            iota_bf[:], pattern=[[1, T]], base=0, channel_multiplier=0,
            allow_small_or_imprecise_dtypes=True,
        )
        iota_f = meta.tile([128, T], dt.float32, tag="iota_f")
        nc.gpsimd.iota(
            iota_f[:], pattern=[[1, T]], base=0, channel_multiplier=0,
            allow_small_or_imprecise_dtypes=True,
        )

        # tiny dummy activation: hoists the one-time act-table load to t~0
        dummy = meta.tile([128, 1], dt.float32, tag="dummy")
        nc.scalar.activation(
            dummy[:], iota_f[:, 0:1], mybir.ActivationFunctionType.Exp
        )

        G1 = psum.tile([128, T], dt.float32, tag="G1")
        G2 = psum.tile([128, T], dt.float32, tag="G2")

        # ---- main loop ----
        for s in range(NSUP):
            xt = xpool.tile([128, SUPER * T], dt.float32, tag="xt")
            x3 = xt[:].rearrange("p (b t) -> p b t", b=SUPER)
            dsplit = [2, 2, 4, 4, 4] if s == 0 else [8, 8]
            off = 0
            for dn in dsplit:
                nc.sync.dma_start(
                    x3[:, off:off + dn, :],
                    preds3[:, s * SUPER + off: s * SUPER + off + dn, :],
                )
                off += dn
            # softplus(x) = Ln(Exp(x) + 1); both funcs live in the
            # natural_log_exp_and_others table set (no table switch).
            # First super is chunked fine so ACT starts right after the
            # first DMA; last super chunked so PE drains earlier.
            # Each super only processes columns [0, ms) per block, where
            # ms is the max extent of its (descending-sorted) blocks.
            csplit = ([2, 2, 4, 4, 4] if s == 0
                      else ([8, 8] if s == NSUP - 1 else [SUPER]))
            ext = spool.tile([128, SUPER * T], dt.float32, tag="ext")
            spt = spool.tile([128, SUPER * T], dt.bfloat16, tag="spt")
            xb = spool.tile([128, SUPER * T], dt.bfloat16, tag="xb")
            xt3 = xt[:].rearrange("p (b t) -> p b t", b=SUPER)
            ext3 = ext[:].rearrange("p (b t) -> p b t", b=SUPER)
            spt3 = spt[:].rearrange("p (b t) -> p b t", b=SUPER)
            xb3 = xb[:].rearrange("p (b t) -> p b t", b=SUPER)
            coff = 0
            for cn in csplit:
                bsl = slice(coff, coff + cn)
                mc = mb[s * SUPER + coff]      # extent of chunk's first block
                coff += cn
                nc.scalar.activation(
                    ext3[:, bsl, 0:mc], xt3[:, bsl, 0:mc],
                    mybir.ActivationFunctionType.Exp,
                )
                nc.scalar.activation(
                    spt3[:, bsl, 0:mc], ext3[:, bsl, 0:mc],
                    mybir.ActivationFunctionType.Ln, bias=1.0,
                )
            for hh in range(2):
                bsl = slice(hh * (SUPER // 2), (hh + 1) * (SUPER // 2))
                mc = mb[s * SUPER + hh * (SUPER // 2)]
                nc.gpsimd.tensor_copy(xb3[:, bsl, 0:mc], xt3[:, bsl, 0:mc])
            for bs in range(SUPER):
                b = s * SUPER + bs
                m = mb[b]
                pfx1 = ppool.tile([128, T], dt.bfloat16, tag="pfx1")
                nc.vector.tensor_scalar(
                    pfx1[:, 0:m], iota_bf[:, 0:m],
                    kA[:, b:b + 1], alpha[:, b:b + 1],
                    Alu.is_le, Alu.mult,
                )
                pfx2 = ppool.tile([128, T], dt.bfloat16, tag="pfx2")
                nc.vector.tensor_scalar(
                    pfx2[:, 0:m], iota_bf[:, 0:m],
                    kS[:, b:b + 1], alpha[:, b:b + 1],
                    Alu.is_le, Alu.mult,
                )
                sp_blk = spt[:, bs * T:bs * T + m]
                x_blk = xb[:, bs * T:bs * T + m]
                nc.tensor.matmul(
                    G1[0:m, 0:m], lhsT=sp_blk, rhs=pfx1[:, 0:m],
                    start=(b == 0), stop=(b == BLOCKS - 1),
                    skip_group_check=True,
                )
                nc.tensor.matmul(
                    G2[0:m, 0:m], lhsT=x_blk, rhs=pfx2[:, 0:m],
                    start=(b == 0), stop=(b == BLOCKS - 1),
                    skip_group_check=True,
                )

        g1_sb = meta.tile([128, T], dt.float32, tag="g1_sb")
        nc.vector.tensor_copy(g1_sb[:], G1[:])
        g2_sb = meta.tile([128, T], dt.float32, tag="g2_sb")
        nc.vector.tensor_copy(g2_sb[:], G2[:])
        nc.sync.dma_start(g1_out, g1_sb[:])
        nc.sync.dma_start(g2_out, g2_sb[:])

    # Force Exp and Ln to resolve to the single combined table set
    # (natural_log_exp_and_others) instead of alternating exp_and_others /
    # natural_log loads every super-tile. Positions (= set ids) preserved;
    # other sets are emptied so the chooser can't pick them.
    import concourse.bacc as bacc_mod
    orig_tables = bacc_mod.get_activation_tables

    def only_combined(arch):
        out = {}
        for name, fns in orig_tables(arch).items():
            out[name] = fns if name == "natural_log_exp_and_others" else set()
        return out

    bacc_mod.get_activation_tables = only_combined
    try:
        nc.compile()
    finally:
        bacc_mod.get_activation_tables = orig_tables
    return nc


_PROGS = {}


def _get_prog(mb):
    if mb not in _PROGS:
        _PROGS[mb] = build_program(mb)
    return _PROGS[mb]


def make_in_maps(preds, sample_weight, targets_d, targets_e):
    """Shard + sort rows descending by mask extent kA (the loss is
    row-permutation invariant), so block b only needs columns
    [0, mb[b]).  Returns (in_maps, mb) with mb derived exactly from the
    data (max over cores, rounded up to a multiple of 8)."""
    p = np.asarray(preds, dtype=np.float32)
    d = np.clip(np.asarray(targets_d), 0, T - 1).astype(np.int32)
    e = (np.asarray(targets_e) != 0).astype(np.int32)
    sw = np.asarray(sample_weight, dtype=np.float32)
    kA_all = np.where(e == 1, T - 1, d)
    in_maps = []
    blockmax = np.zeros((NCORES, BLOCKS), dtype=np.int64)
    for c in range(NCORES):
        sl = slice(c * NS, (c + 1) * NS)
        order = np.argsort(-kA_all[sl], kind="stable")
        # rank q = b*128 + p  ->  shard position r = p*128 + b
        Q = order.reshape(BLOCKS, 128)        # Q[b, p] = source row of rank
        src_rows = Q.T                         # [p, b]
        blockmax[c] = kA_all[sl][Q[:, 0]]      # descending: rank b*128 is max
        flat = src_rows.reshape(-1)            # r = p*128 + b order
        in_maps.append({
            "preds": np.ascontiguousarray(p[sl][flat]),
            "d": np.ascontiguousarray(d[sl][src_rows]),
            "e": np.ascontiguousarray(e[sl][src_rows]),
            "sw": np.ascontiguousarray(sw[sl][src_rows]),
        })
    mb = blockmax.max(axis=0) + 1
    mb = np.minimum(((mb + 7) // 8) * 8, T)
    mb = np.maximum.accumulate(mb[::-1])[::-1]   # enforce non-increasing
    mb[0] = T                                    # block 0 resets full PSUM
    return in_maps, tuple(int(v) for v in mb)


def kernel(preds, weight, sample_weight, targets_d, targets_e):
    global LAST_RESULTS
    in_maps, mb = make_in_maps(preds, sample_weight, targets_d, targets_e)
    prog = _get_prog(mb)
    trace = bool(int(os.environ.get("SURV_TRACE", "0")))
    res = None
    last_err = None
    for attempt in range(3):
        try:
            res = run_bass_kernel_spmd(
                prog, in_maps, list(range(NCORES)), trace=trace
            )
            break
        except Exception as ex:  # transient NRT/device errors: retry
            last_err = ex
            import time as _time
            _time.sleep(2.0 * (attempt + 1))
    if res is None:
        raise last_err
    LAST_RESULTS = res
    w64 = np.asarray(weight, dtype=np.float64)
    num = 0.0
    for c in range(NCORES):
        g1 = res.results[c]["g1"].astype(np.float64)
        g2 = res.results[c]["g2"].astype(np.float64)
        num += float((np.diagonal(g1) - np.diagonal(g2)) @ w64)
    den = float(np.asarray(sample_weight, dtype=np.float64).sum())
    return np.float32(num / max(den, EPS))



# revision 31
# speedup vs baseline: 1.0302x; 1.0302x over previous
"""BCE survival loss on 8 trn2 NeuronCores.

Math (per row i of preds [N,T], d=clip(targets_d,0,T-1), e=targets_e!=0):
  bce = softplus(x) - y*x   with y = [j < d+(1-e)], mask = e ? ones : [j <= d]
  per_sample = sum_j w_j*mask*bce / mth,  mth = e ? T : d+1
  out = sum_i sw_i*per_sample_i / max(sum_i sw_i, eps)

Split by row type (alpha_i = sw_i/mth_i, kS_i = d_i - e_i):
  censored (e=0): y == mask == [j <= d], so masked bce = [j<=d]*(sp - x)
                  = [j<=d]*softplus(-x): activation with scale=-1,
                  no x copy, one masked matmul into GC (split hi/lo, below).
  event (e=1):    mask is all-ones: sum_j w_j sp_ij needs NO mask -> rank-1
                  matmul (rhs = alpha column, 1 moving row) into AE[j];
                  minus [j<=kS]*x masked matmul into G2E.
  general blocks (mixed rows, cross-core padding): baseline scheme
                  [j<=kA]*alpha*sp into GChi and [j<=kS]*alpha*x into G2E.

  NUM = sum_j w_j*(AE[j] + diag(GChi)[j] + diag(GClo)[j]|j<64 - diag(G2E)[j])
  out = NUM / sum(sw).

Rows are host-sorted into [general | events by d desc | censored by d desc]
blocks of 128 (rank q = b*128+p -> partition p, block b), and preds are
host-packed to only the columns each block actually reads: events/general
need all T cols, censored only [0, max d+1). The packed [128, TOT] layout
keeps every DMA fully contiguous per partition.

The censored GC accumulator is split at the first block whose extent drops
to <=64 (bumped to exactly 64): earlier blocks accumulate into GChi
[128,128], later ones into GClo [64,64], so GChi/AE/G2E are all read out
mid-stream and only the tiny GClo readout sits on the tail. Output DMAs go
through the Pool SWDGE queue (cheap trigger after the data sem) instead of
the SP HWDGE path (565ns seq + ~1.3us DGE latency after the sem).

Only diagonals / AE are used; host does the final tiny reduction.
"""

import os
from contextlib import ExitStack

import numpy as np

import concourse.bacc as bacc
import concourse.bass as bass
import concourse.mybir as mybir
import concourse.tile as tile
from concourse.bass_utils import run_bass_kernel_spmd

dt = mybir.dt
Alu = mybir.AluOpType
Act = mybir.ActivationFunctionType

N, T = 131072, 128
NCORES = 8
NS = N // NCORES          # rows per core shard = 16384
BLOCKS = NS // 128        # 128 row-blocks per core
MACRO = 32                # blocks per macro-tile (DMA/ACT granularity)
NMAC = BLOCKS // MACRO    # 4 macro-tiles
EPS = 1e-9
GEN, EVT, CEN = 0, 1, 2

LAST_RESULTS = None       # BassKernelResults of the most recent run (for test.py)


def build_program(meta):
    """meta = (btype, dext, sext): per-block type, packed/DMA/ACT column
    extent, and masked-matmul extent (all len BLOCKS; extents multiples
    of 8; events/general have dext=128)."""
    btype, dext, sext = meta
    boff = [0]
    for b in range(BLOCKS):
        boff.append(boff[-1] + dext[b])
    TOT = boff[-1]
    # pad the packed width to a multiple of 128 cols so the per-partition
    # DRAM stride is 512B-aligned (odd strides mis-address on real HW)
    TOTP = -(-TOT // 128) * 128

    ae_w = [b for b in range(BLOCKS) if btype[b] == EVT]
    g2_w = [b for b in range(BLOCKS)
            if btype[b] == GEN or (btype[b] == EVT and sext[b] > 0)]
    # censored GC split: hi chain while sext > 64, lo chain once == 64
    gchi_w = [b for b in range(BLOCKS)
              if btype[b] == GEN or (btype[b] == CEN and sext[b] > 64)]
    gclo_w = [b for b in range(BLOCKS) if btype[b] == CEN and sext[b] <= 64]
    assert g2_w and gchi_w and ae_w
    assert btype[g2_w[0]] == GEN and btype[gchi_w[0]] == GEN, \
        "first PSUM writer must be a full-extent general block"
    assert not gclo_w or sext[gclo_w[0]] == 64, \
        "first GClo writer must have extent exactly 64"

    nc = bacc.Bacc(
        "TRN2", target_bir_lowering=False, debug=False, num_devices=NCORES
    )
    xpk = nc.dram_tensor("xpk", [128, TOTP], dt.float32, kind="ExternalInput").ap()
    m_in = nc.dram_tensor(
        "meta3", [128, 3 * BLOCKS], dt.float32, kind="ExternalInput").ap()
    gc_out = nc.dram_tensor("gc", [128, T], dt.float32, kind="ExternalOutput").ap()
    gl_out = nc.dram_tensor("gl", [64, 64], dt.float32, kind="ExternalOutput").ap()
    g2_out = nc.dram_tensor("g2", [128, T], dt.float32, kind="ExternalOutput").ap()
    ae_out = nc.dram_tensor("ae", [128, 1], dt.float32, kind="ExternalOutput").ap()

    with ExitStack() as ctx:
        tc = ctx.enter_context(tile.TileContext(nc))
        xpool = ctx.enter_context(tc.tile_pool(name="x", bufs=4))
        spool = ctx.enter_context(tc.tile_pool(name="sp", bufs=2))
        bpool = ctx.enter_context(tc.tile_pool(name="xb", bufs=2))
        # one pfx buffer per block: never reused, so TileContext emits no
        # WAR EventSemaphore waits on the DVE SEQ (70ns of dispatch each)
        n_pe = sum(1 for b in range(BLOCKS)
                   if btype[b] == EVT and sext[b] > 0) + 1
        n_pc = sum(1 for b in range(BLOCKS) if btype[b] == CEN) + 1
        n_pg = sum(1 for b in range(BLOCKS) if btype[b] == GEN) + 1
        ppool_e = ctx.enter_context(tc.tile_pool(name="pfxe", bufs=n_pe))
        ppool_c = ctx.enter_context(tc.tile_pool(name="pfxc", bufs=n_pc))
        ppool_g = ctx.enter_context(tc.tile_pool(name="pfxg", bufs=2 * n_pg))
        meta_p = ctx.enter_context(tc.tile_pool(name="meta", bufs=1))
        psum = ctx.enter_context(tc.tile_pool(name="acc", bufs=1, space="PSUM"))

        # iota first so the act-table-load hoist (dummy) fires immediately
        iota_bf = meta_p.tile([128, T], dt.bfloat16, tag="iota_bf")
        nc.gpsimd.iota(
            iota_bf[:], pattern=[[1, T]], base=0, channel_multiplier=0,
            allow_small_or_imprecise_dtypes=True,
        )
        # tiny dummy activation: hoists the one-time act-table load to t~0
        dummy = meta_p.tile([128, 1], dt.float32, tag="dummy")
        nc.scalar.activation(dummy[:], iota_bf[:, 0:1], Act.Exp)

        # metadata (df|ef|sw as one f32 tensor) via the Pool SWDGE queue
        # so the x loads own the HWDGE path; MUST be emitted before the DVE
        # prep chain that reads m_t (trace order = dependency order)
        m_t = meta_p.tile([128, 3 * BLOCKS], dt.float32, tag="m_t")
        nc.gpsimd.dma_start(m_t[:], m_in)
        df = m_t[:, 0:BLOCKS]
        ef = m_t[:, BLOCKS:2 * BLOCKS]
        sw_t = m_t[:, 2 * BLOCKS:3 * BLOCKS]

        # tsum = d + 200*e ; kA = min(tsum,127) ; mth = min(tsum+1,128)
        tsum = meta_p.tile([128, BLOCKS], dt.float32, tag="tsum")
        nc.vector.tensor_scalar(tsum[:], ef, 200.0, None, Alu.mult)
        nc.vector.tensor_add(tsum[:], tsum[:], df)
        kA = meta_p.tile([128, BLOCKS], dt.float32, tag="kA")
        nc.vector.tensor_scalar(kA[:], tsum[:], 127.0, None, Alu.min)
        mth = meta_p.tile([128, BLOCKS], dt.float32, tag="mth")
        nc.vector.tensor_scalar(mth[:], tsum[:], 1.0, 128.0, Alu.add, Alu.min)
        kS = meta_p.tile([128, BLOCKS], dt.float32, tag="kS")
        nc.vector.tensor_sub(kS[:], df, ef)
        rec = meta_p.tile([128, BLOCKS], dt.float32, tag="rec")
        nc.vector.reciprocal(rec[:], mth[:])
        alpha = meta_p.tile([128, BLOCKS], dt.float32, tag="alpha")
        nc.vector.tensor_mul(alpha[:], sw_t, rec[:])
        alpha_bf = meta_p.tile([128, BLOCKS], dt.bfloat16, tag="alpha_bf")
        nc.vector.tensor_copy(alpha_bf[:], alpha[:])

        AE = psum.tile([128, 1], dt.float32, tag="AE")
        GChi = psum.tile([128, T], dt.float32, tag="GChi")
        GClo = psum.tile([64, 64], dt.float32, tag="GClo")
        G2 = psum.tile([128, T], dt.float32, tag="G2")

        def emit_block(b, sp, xb, loff):
            s = sext[b]
            if btype[b] == EVT:
                nc.tensor.matmul(
                    AE[:, 0:1], lhsT=sp[:, loff:loff + 128],
                    rhs=alpha_bf[:, b:b + 1],
                    start=(b == ae_w[0]), stop=(b == ae_w[-1]),
                    skip_group_check=True,
                )
                if s > 0:
                    pfx = ppool_e.tile([128, T], dt.bfloat16, tag="pfx2")
                    nc.vector.tensor_scalar(
                        pfx[:, 0:s], iota_bf[:, 0:s],
                        kS[:, b:b + 1], alpha[:, b:b + 1],
                        Alu.is_le, Alu.mult,
                    )
                    nc.tensor.matmul(
                        G2[0:s, 0:s], lhsT=xb[:, loff:loff + s],
                        rhs=pfx[:, 0:s],
                        start=(b == g2_w[0]), stop=(b == g2_w[-1]),
                        skip_group_check=True,
                    )
            elif btype[b] == GEN:
                pfx1 = ppool_g.tile([128, T], dt.bfloat16, tag="pfx1")
                nc.vector.tensor_scalar(
                    pfx1[:], iota_bf[:],
                    kA[:, b:b + 1], alpha[:, b:b + 1],
                    Alu.is_le, Alu.mult,
                )
                nc.tensor.matmul(
                    GChi[:], lhsT=sp[:, loff:loff + 128], rhs=pfx1[:],
                    start=(b == gchi_w[0]), stop=(b == gchi_w[-1]),
                    skip_group_check=True,
                )
                pfx2 = ppool_g.tile([128, T], dt.bfloat16, tag="pfx2g")
                nc.vector.tensor_scalar(
                    pfx2[:], iota_bf[:],
                    kS[:, b:b + 1], alpha[:, b:b + 1],
                    Alu.is_le, Alu.mult,
                )
                nc.tensor.matmul(
                    G2[:], lhsT=xb[:, loff:loff + 128], rhs=pfx2[:],
                    start=(b == g2_w[0]), stop=(b == g2_w[-1]),
                    skip_group_check=True,
                )
            else:  # CEN: sp holds softplus(-x); mask [j<=d] via kS (=d here)
                pfx = ppool_c.tile([128, T], dt.bfloat16, tag="pfxc")
                nc.vector.tensor_scalar(
                    pfx[:, 0:s], iota_bf[:, 0:s],
                    kS[:, b:b + 1], alpha[:, b:b + 1],
                    Alu.is_le, Alu.mult,
                )
                if s > 64:
                    nc.tensor.matmul(
                        GChi[0:s, 0:s], lhsT=sp[:, loff:loff + s],
                        rhs=pfx[:, 0:s],
                        start=(b == gchi_w[0]), stop=(b == gchi_w[-1]),
                        skip_group_check=True,
                    )
                else:
                    nc.tensor.matmul(
                        GClo[0:s, 0:s], lhsT=sp[:, loff:loff + s],
                        rhs=pfx[:, 0:s],
                        start=(b == gclo_w[0]), stop=(b == gclo_w[-1]),
                        skip_group_check=True,
                    )

        # PSUM->SBUF copies happen as soon as each chain stops (cheap DVE
        # op, overlaps the stream); the output DMAs are all emitted at the
        # very end of the SP stream so their SEQ-side sem waits can never
        # delay an x-load dispatch.
        out_dmas = []

        def readout(tag, src, dst, shape, eng="v"):
            # eng="s" puts the PSUM->SBUF copy on the ACT engine: the late
            # gc/gl copies block on their stop-matmuls, and parking them on
            # the DVE would hold back the DVE engine-tick sem that the
            # earlier ae/g2 output DMAs wait on (oversync).
            sb = meta_p.tile(list(shape), dt.float32, tag=tag)
            if eng == "v":
                nc.vector.tensor_copy(sb[:], src)
            else:
                nc.scalar.copy(sb[:], src)
            out_dmas.append((dst, sb))

        # ---- main loop over macro-tiles of 32 blocks ----
        for mi in range(NMAC):
            b0 = mi * MACRO
            PS = boff[b0 + MACRO] - boff[b0]
            xt = xpool.tile([128, PS], dt.float32, tag="xt")
            dsplit = ([4, 4, 8, 16] if mi == 0 else [16, 16])
            off = 0
            for dn in dsplit:
                lo = boff[b0 + off] - boff[b0]
                hi = boff[b0 + off + dn] - boff[b0]
                nc.sync.dma_start(
                    xt[:, lo:hi], xpk[:, boff[b0 + off]:boff[b0 + off + dn]]
                )
                off += dn
            # softplus(s*x) = Ln(Exp(s*x) + 1), s = -1 for censored blocks
            # (their masked bce is softplus(-x); events/general use +1).
            # Both funcs live in the natural_log_exp_and_others table set
            # (no table switch). Exp chunks are coarse (nothing waits on
            # them but the Ln); Ln chunks are fine over censored blocks so
            # their GC matmuls trail the Ln stream closely, and emit_block
            # is interleaved per Ln run so readout copies (ACT engine) sit
            # right after the Ln that unblocks their stop-matmul.
            ext = spool.tile([128, PS], dt.float32, tag="ext")
            sp = spool.tile([128, PS], dt.bfloat16, tag="sp")
            # bf16 x copy for event/general blocks only. They are a prefix
            # of the macro with uniform dext=128, so a strided 3D view of
            # that prefix batches blocks per copy.
            evgen = [b for b in range(b0, b0 + MACRO) if btype[b] != CEN]
            xb = None
            if evgen:
                nev = len(evgen)
                assert evgen == list(range(b0, b0 + nev))
                assert all(dext[b] == 128 for b in evgen)
                xb = bpool.tile([128, PS], dt.bfloat16, tag="xb")
                xt3 = xt[:, 0:nev * 128].rearrange("p (b t) -> p b t", t=128)
                xb3 = xb[:, 0:nev * 128].rearrange("p (b t) -> p b t", t=128)
                # group consecutive ev/gen blocks into copy chunks of <=8
                chunks = []
                for b in evgen:
                    mc = 128 if btype[b] == GEN else sext[b]
                    if mc == 0:
                        continue
                    if (chunks and b == chunks[-1][-1][0] + 1
                            and len(chunks[-1]) < 8):
                        chunks[-1].append((b, mc))
                    else:
                        chunks.append([(b, mc)])
                for ch in chunks:
                    mc = max(m for _, m in ch)
                    blo, bhi = ch[0][0], ch[-1][0]
                    nc.gpsimd.tensor_copy(
                        xb3[:, blo - b0:bhi - b0 + 1, 0:mc],
                        xt3[:, blo - b0:bhi - b0 + 1, 0:mc],
                    )
            # Per DMA-sized chunk: one Exp run set, then Ln runs (censored
            # split to 8 blocks), each Ln followed by its blocks' matmuls
            # and any readout whose chain just stopped. Interleaving Exp
            # and Ln per chunk lets the ACT engine run Ln of chunk k while
            # chunk k+1 is still in flight on the DMA.
            lsplit = [4, 4, 8, 16] if mi == 0 else [16, 16]
            coff = 0
            for cn in lsplit:
                runs = []
                for b in range(b0 + coff, b0 + coff + cn):
                    is_c = btype[b] == CEN
                    if runs and runs[-1][0] == is_c:
                        runs[-1][1].append(b)
                    else:
                        runs.append((is_c, [b]))
                coff += cn
                for is_c, bs in runs:
                    lo = boff[bs[0]] - boff[b0]
                    hi = boff[bs[-1] + 1] - boff[b0]
                    nc.scalar.activation(
                        ext[:, lo:hi], xt[:, lo:hi], Act.Exp,
                        scale=(-1.0 if is_c else 1.0),
                    )
                lruns = []
                for is_c, bs in runs:
                    if is_c:
                        lruns.extend(
                            bs[k:k + 8] for k in range(0, len(bs), 8))
                    else:
                        lruns.append(bs)
                for bs in lruns:
                    lo = boff[bs[0]] - boff[b0]
                    hi = boff[bs[-1] + 1] - boff[b0]
                    nc.scalar.activation(
                        sp[:, lo:hi], ext[:, lo:hi], Act.Ln, bias=1.0,
                    )
                    for b in bs:
                        emit_block(b, sp, xb, boff[b] - boff[b0])
                        # read out each accumulator as soon as its chain
                        # stops so the copies/DMAs overlap the tail
                        if b == ae_w[-1]:
                            readout("ae_sb", AE[:, 0:1], ae_out, (128, 1))
                        if b == g2_w[-1]:
                            readout("g2_sb", G2[:], g2_out, (128, T))
                        if b == gchi_w[-1]:
                            readout("gc_sb", GChi[:], gc_out, (128, T),
                                    eng="s")

        if gclo_w:
            readout("gl_sb", GClo[:], gl_out, (64, 64), eng="s")
        for dst, sb in out_dmas:
            nc.sync.dma_start(dst, sb[:])

    # Force Exp and Ln to resolve to the single combined table set
    # (natural_log_exp_and_others) instead of alternating exp_and_others /
    # natural_log loads every macro-tile. Positions (= set ids) preserved;
    # other sets are emptied so the chooser can't pick them.
    import concourse.bacc as bacc_mod
    orig_tables = bacc_mod.get_activation_tables

    def only_combined(arch):
        out = {}
        for name, fns in orig_tables(arch).items():
            out[name] = fns if name == "natural_log_exp_and_others" else set()
        return out

    bacc_mod.get_activation_tables = only_combined
    try:
        nc.compile()
    finally:
        bacc_mod.get_activation_tables = orig_tables
    return nc


_PROGS = {}


def _get_prog(meta):
    if meta not in _PROGS:
        _PROGS[meta] = build_program(meta)
    return _PROGS[meta]


def _r8(v):
    return int(min((int(v) + 7) // 8 * 8, T))


def make_in_maps(preds, sample_weight, targets_d, targets_e):
    """Sort rows [general | events by d desc | censored by d desc] (the
    loss is row-permutation invariant), pack preds to the per-block
    column extents, and build the per-core input maps plus the
    compile-time structure tuple."""
    p = np.asarray(preds, dtype=np.float32)
    d = np.clip(np.asarray(targets_d), 0, T - 1).astype(np.int32)
    e = (np.asarray(targets_e) != 0).astype(np.int32)
    sw = np.asarray(sample_weight, dtype=np.float32)

    orders, ecnt = [], []
    for c in range(NCORES):
        sl = slice(c * NS, (c + 1) * NS)
        dl, el = d[sl], e[sl]
        ev = np.where(el == 1)[0]
        cs = np.where(el == 0)[0]
        ev = ev[np.argsort(-dl[ev], kind="stable")]
        cs = cs[np.argsort(-dl[cs], kind="stable")]
        orders.append(np.concatenate([ev, cs]))
        ecnt.append(len(ev))
    nE = min(ecnt) // 128
    nCs = -(-max(ecnt) // 128)          # ceil
    if nCs == nE:                        # ensure >=1 general block up front
        nE -= 1
    nG = nCs - nE
    assert nG >= 1 and nE >= 1
    # final block order: [general ranks | event ranks | censored ranks]
    rank_of_block = ([*range(nE, nCs)] + [*range(nE)] + [*range(nCs, BLOCKS)])
    btype = tuple([GEN] * nG + [EVT] * nE + [CEN] * (BLOCKS - nCs))

    dext = np.zeros(BLOCKS, dtype=np.int64)
    sext = np.zeros(BLOCKS, dtype=np.int64)
    Q2s = []
    for c in range(NCORES):
        sl = slice(c * NS, (c + 1) * NS)
        dl = d[sl]
        Q2 = np.empty((BLOCKS, 128), dtype=np.int64)
        for b in range(BLOCKS):
            rb = rank_of_block[b]
            Q2[b] = orders[c][rb * 128:(rb + 1) * 128]
            dmax = int(dl[Q2[b]].max())
            if btype[b] == EVT:
                sext[b] = max(sext[b], dmax)      # kS+1 = d
            elif btype[b] == CEN:
                dext[b] = max(dext[b], dmax + 1)
        Q2s.append(Q2)
    dext = [128 if btype[b] != CEN else _r8(dext[b]) for b in range(BLOCKS)]
    sext = [128 if btype[b] == GEN else
            (_r8(sext[b]) if btype[b] == EVT else dext[b])
            for b in range(BLOCKS)]
    # bump the first censored block that would drop below 64 up to exactly
    # 64 so it can open (reset) the GClo accumulation region
    for b in range(BLOCKS):
        if btype[b] == CEN and sext[b] <= 64:
            dext[b] = sext[b] = 64
            break
    meta = (btype, tuple(dext), tuple(sext))

    in_maps = []
    for c in range(NCORES):
        sl = slice(c * NS, (c + 1) * NS)
        Q2 = Q2s[c]
        X = p[sl][Q2]                       # [BLOCKS, 128p, T]
        xpk = np.concatenate(
            [X[b, :, 0:dext[b]] for b in range(BLOCKS)], axis=1)
        totp = -(-xpk.shape[1] // 128) * 128
        xpk = np.pad(xpk, [(0, 0), (0, totp - xpk.shape[1])])
        meta3 = np.concatenate(
            [d[sl][Q2].T.astype(np.float32),
             e[sl][Q2].T.astype(np.float32),
             sw[sl][Q2].T], axis=1)
        in_maps.append({
            "xpk": np.ascontiguousarray(xpk),
            "meta3": np.ascontiguousarray(meta3),
        })
    return in_maps, meta


def kernel(preds, weight, sample_weight, targets_d, targets_e):
    global LAST_RESULTS
    in_maps, meta = make_in_maps(preds, sample_weight, targets_d, targets_e)
    prog = _get_prog(meta)
    trace = bool(int(os.environ.get("SURV_TRACE", "0")))
    res = None
    last_err = None
    for attempt in range(3):
        try:
            res = run_bass_kernel_spmd(
                prog, in_maps, list(range(NCORES)), trace=trace
            )
            break
        except Exception as ex:  # transient NRT/device errors: retry
            last_err = ex
            import time as _time
            _time.sleep(2.0 * (attempt + 1))
    if res is None:
        raise last_err
    LAST_RESULTS = res
    btype, _, sext = meta
    has_lo = any(btype[b] == CEN and sext[b] <= 64 for b in range(BLOCKS))
    w64 = np.asarray(weight, dtype=np.float64)
    num = 0.0
    for c in range(NCORES):
        r = res.results[c]
        dg = r["gc"].astype(np.float64).diagonal().copy()
        if has_lo:
            dg[:64] += r["gl"].astype(np.float64).diagonal()
        dg += r["ae"].astype(np.float64)[:, 0]
        dg -= r["g2"].astype(np.float64).diagonal()
        num += float(dg @ w64)
    den = float(np.asarray(sample_weight, dtype=np.float64).sum())
    return np.float32(num / max(den, EPS))
